# revision 1
# baseline (speedup 1.0000x reference)
"""Chamfer distance (CDLoss) Trainium2 Bass kernel.

Problem: B=8, N=4096, D=3.
  T[b,i,j] = ||pred[b,i] - gt[b,j]||^2
  loss = (sum_bj min_i T + sum_bi min_j T) / B

Sharding: one batch per NeuronCore (8 cores, SPMD). Each core computes
  partial_b[p] (per-partition sums of min distances for batch b)
and the host adds the 128 values per core, sums cores, divides by B.

Per-core algorithm (flash-style; the NxN matrix never leaves PSUM):
  Two symmetric passes; pass A puts pred-index i on PSUM partitions and
  gt-index j on the free axis, pass B swaps roles, so both min
  directions are free-axis reductions. Each [128,512] PSUM tile comes
  from ONE matmul with an augmented K=24 contraction that yields the
  full squared distance directly:
     T[i,j] = -2*p_i.g_j + ||g_j||^2 + ||p_i||^2
  rows 0-17: bf16 hi/mid/lo splits of the coordinates (6 cross terms
  x 3 dims; three bf16 levels carry ~24 mantissa bits -> fp32-grade
  dot products while the PE runs at full bf16 rate; fp32 matmul would
  be 4x slower); rows 18-20: ones x moving-side norm h/m/l; rows
  21-23: stationary-side norm h/m/l x ones. End-to-end relative error
  vs the fp32 reference: ~6e-6.

  The PSUM drain is the bottleneck (DVE reduce runs at 1 elem/lane/cyc
  at 0.96 GHz, a DVE op may read at most one PSUM operand, and GPSIMD
  has no PSUM port), so the drain is split between the two engines
  that can read PSUM: per 128-row tile (8 PSUM banks of distances),
  VectorE min-reduces 2 banks directly in fp32 while ScalarE copies 6
  banks to fp16 in SBUF at 1.2 GHz (distances are small positive
  values, so fp16 keeps ~2.4e-4 relative on the candidates); VectorE
  then folds the fp16 staging 3072->1536->768->384 with 2x-mode
  tensor_tensor mins and min-reduces the remainder. Engine busy per
  row-tile: DVE ~3.2us, ACT ~3.0us, PE ~1.7us (overlapped).

  Preprocessing builds the S (stationary) and R (moving) operand
  tensors [16, 4096] bf16 via per-row partition-flatten DMAs spread
  over the SP/ACT HWDGE queues plus a SWDGE queue; pass-A operands are
  emitted first so the main loop starts while pass-B rows stream in.
"""

import numpy as np

import concourse.bacc as bacc
import concourse.bass as bass
import concourse.tile as tile
from concourse import mybir
from concourse.bass_utils import run_bass_kernel_spmd

N = 4096
D = 3
B = 8
P = 128            # SBUF/PSUM partitions
KP = N // P        # 32 points per partition in the staging layout
NT = N // P        # 32 row-tiles per pass
CH = 512           # matmul moving free dim (one PSUM bank of fp32)
QF = 1024          # psum tile free size (2 banks); 4 tiles in flight
KROWS = 24         # augmented contraction rows

f32 = mybir.dt.float32
f16 = mybir.dt.float16
bf16 = mybir.dt.bfloat16

TRACE = False
LAST_RESULT = None

_nc_cache = None


def _build_bass():
    # Bacc (not raw Bass): its compile() legalizes multi-wait joins into
    # event semaphores; the TPB ISA has a single wait slot per instruction.
    nc = bacc.Bacc(
        "TRN2", target_bir_lowering=False, debug=False, num_devices=B,
        num_swdge_queues=4,
    )
    pred = nc.declare_dram_parameter("prediction", [N, D], f32, isOutput=False)
    gt = nc.declare_dram_parameter("ground_truth", [N, D], f32, isOutput=False)
    # per-partition partial sums; host adds the 128 values per core
    out_dram = nc.declare_dram_parameter("partial", [P, 1], f32, isOutput=True)

    with tile.TileContext(nc) as tc:
        with (
            tc.tile_pool(name="singles", bufs=1) as singles,
            tc.tile_pool(name="work", bufs=2) as work,
            tc.tile_pool(name="stage", bufs=3) as stage,
            tc.tile_pool(name="folds", bufs=3) as folds,
            tc.tile_pool(name="psum", bufs=4, space="PSUM") as psum,
        ):
            # ---------- preprocessing ----------
            # Build, per input tensor X:
            #   S_X [14, 4096] bf16 : stationary rows  [xh xh xl xl 1 1]
            #   R_X [14, 4096] bf16 : moving rows [-2xh -2xl -2xh -2xl nh nl]
            #   nsum [128, 1] f32   : per-partition sum of ||x||^2
            # Row pairing: sum_r S_P[r,i]*R_G[r,j] =
            #   -2*(ph+pl).(gh+gl) + (nh+nl) = -2 p.g + ||g||^2 (to ~2^-18)
            def levels(xdram, tag):
                xt = work.tile([P, KP, D], f32, tag="xt")
                nc.sync.dma_start(
                    out=xt, in_=xdram[:].rearrange("(p k) d -> p k d", p=P)
                )
                # inner [k d] -> [d k] so per-(level,dim) rows are contiguous
                # 32-element runs for the flatten DMAs below
                xr = work.tile([P, D, KP], f32, tag="xr")
                nc.vector.tensor_copy(out=xr, in_=xt[:].rearrange("p k d -> p d k"))
                # bf16 hi/mid/lo decomposition (3 levels carry ~24
                # mantissa bits -> fp32-equivalent dot products)
                def split3(val, pfx, shape):
                    h = work.tile(shape, bf16, tag=f"{pfx}h")
                    nc.vector.tensor_copy(out=h, in_=val)
                    h32 = work.tile(shape, f32, tag=f"{pfx}h32")
                    nc.vector.tensor_copy(out=h32, in_=h)
                    r1 = work.tile(shape, f32, tag=f"{pfx}r1")
                    nc.vector.tensor_sub(r1, val, h32)
                    m = work.tile(shape, bf16, tag=f"{pfx}m")
                    nc.vector.tensor_copy(out=m, in_=r1)
                    m32 = work.tile(shape, f32, tag=f"{pfx}m32")
                    nc.vector.tensor_copy(out=m32, in_=m)
                    r2 = work.tile(shape, f32, tag=f"{pfx}r2")
                    nc.vector.tensor_sub(r2, r1, m32)
                    l = work.tile(shape, bf16, tag=f"{pfx}l")
                    nc.vector.tensor_copy(out=l, in_=r2)
                    return h, m, l

                xh, xm, xl = split3(xr, "x", [P, D, KP])
                # scaled (-2) variants for the moving side
                xhm = work.tile([P, D, KP], bf16, tag="xhm")
                nc.vector.tensor_scalar_mul(xhm, xh, -2.0)
                xmm = work.tile([P, D, KP], bf16, tag="xmm")
                nc.vector.tensor_scalar_mul(xmm, xm, -2.0)
                xlm = work.tile([P, D, KP], bf16, tag="xlm")
                nc.vector.tensor_scalar_mul(xlm, xl, -2.0)
                # squared norms in fp32, then 3-level bf16 split
                sq = work.tile([P, D, KP], f32, tag="sq")
                nc.vector.tensor_mul(sq, xr, xr)
                n32 = work.tile([P, KP], f32, tag="n32")
                nc.vector.tensor_add(n32, sq[:, 0, :], sq[:, 1, :])
                nc.vector.tensor_add(n32, n32, sq[:, 2, :])
                nh, nm, nl = split3(n32, "n", [P, KP])
                return dict(xh=xh, xm=xm, xl=xl, xhm=xhm, xmm=xmm,
                            xlm=xlm, nh=nh, nm=nm, nl=nl)

            flat_engines = [nc.sync, nc.scalar, nc.gpsimd]
            flat_i = [0]

            def flat(dst, r, src2d):
                # [128, 32] staging -> one 4096-wide row (col = p*32+k),
                # round-robin across the two HWDGE queues
                eng = flat_engines[flat_i[0] % len(flat_engines)]
                flat_i[0] += 1
                eng.dma_start(
                    out=dst[r : r + 1, :].rearrange("r (p k) -> r p k", p=P),
                    in_=src2d,
                )

            ones32 = singles.tile([P, KP], bf16, tag="ones32")
            nc.vector.memset(ones32, 1.0)

            def rowcopy(dst, r0, r1, src_r0):
                # duplicate already-flattened rows (contiguous, DMA-cheap)
                eng = flat_engines[flat_i[0] % len(flat_engines)]
                flat_i[0] += 1
                eng.dma_start(
                    out=dst[r0:r1, :], in_=dst[src_r0 : src_r0 + (r1 - r0), :]
                )

            # Row pairing (S[r] * R[r] summed over r = full distance):
            #   0-2:(h,-2h) 3-5:(h,-2m) 6-8:(m,-2h) 9-11:(h,-2l)
            #   12-14:(l,-2h) 15-17:(m,-2m)  [ml/lm/ll dropped, ~2^-27]
            #   18-20:(1, n_hml)  21-23:(n_hml, 1)
            def flats_S(S, lv):
                for d in range(D):
                    flat(S, 0 + d, lv["xh"][:, d, :])
                    flat(S, 6 + d, lv["xm"][:, d, :])
                    flat(S, 12 + d, lv["xl"][:, d, :])
                flat(S, 18, ones32)
                flat(S, 19, ones32)
                flat(S, 20, ones32)
                flat(S, 21, lv["nh"])
                flat(S, 22, lv["nm"])
                flat(S, 23, lv["nl"])
                rowcopy(S, 3, 6, 0)
                rowcopy(S, 9, 12, 0)
                rowcopy(S, 15, 18, 6)

            def flats_R(R, lv):
                for d in range(D):
                    flat(R, 0 + d, lv["xhm"][:, d, :])
                    flat(R, 3 + d, lv["xmm"][:, d, :])
                    flat(R, 9 + d, lv["xlm"][:, d, :])
                flat(R, 18, lv["nh"])
                flat(R, 19, lv["nm"])
                flat(R, 20, lv["nl"])
                flat(R, 21, ones32)
                flat(R, 22, ones32)
                flat(R, 23, ones32)
                rowcopy(R, 6, 9, 0)
                rowcopy(R, 12, 15, 0)
                rowcopy(R, 15, 18, 3)

            lvP = levels(pred, "p")
            lvG = levels(gt, "g")
            S_P = singles.tile([KROWS, N], bf16, tag="S_p")
            R_P = singles.tile([KROWS, N], bf16, tag="R_p")
            S_G = singles.tile([KROWS, N], bf16, tag="S_g")
            R_G = singles.tile([KROWS, N], bf16, tag="R_g")
            # pass-A operands first so the main loop starts while the
            # pass-B flats still stream in the background
            flats_S(S_P, lvP)
            flats_R(R_G, lvG)
            flats_S(S_G, lvG)
            flats_R(R_P, lvP)

            # per-pass rowmin collectors (column it = rowmin of row-tile it)
            Md_A = singles.tile([P, NT], f32, tag="Md_A")  # fp32 direct part
            Mb_A = singles.tile([P, NT], f32, tag="Mb_A")  # fp16 staged part
            Md_B = singles.tile([P, NT], f32, tag="Md_B")
            Mb_B = singles.tile([P, NT], f32, tag="Mb_B")

            # ---------- main passes ----------
            # Per row-tile (8 PSUM banks of distances): DVE min-reduces 2
            # banks directly in fp32; ACT copies 6 banks to fp16 in SBUF
            # (distances are small positive values, so fp16 keeps ~2.4e-4
            # relative); DVE folds the staging with 2x-mode fp16 mins.
            for Md, Mb, S, R in (
                (Md_A, Mb_A, S_P, R_G),
                (Md_B, Mb_B, S_G, R_P),
            ):
                for it in range(NT):
                    lhsT = S[0:KROWS, it * P : (it + 1) * P]

                    def mm_tile(c0):
                        T = psum.tile([P, QF], f32, tag="psumT")
                        for h in range(2):
                            nc.tensor.matmul(
                                T[:, h * CH : (h + 1) * CH],
                                lhsT,
                                R[0:KROWS, (c0 + h) * CH : (c0 + h + 1) * CH],
                                start=True,
                                stop=True,
                            )
                        return T

                    t0 = mm_tile(0)
                    nc.vector.tensor_reduce(
                        out=Md[:, it : it + 1], in_=t0,
                        axis=mybir.AxisListType.X, op=mybir.AluOpType.min,
                    )
                    C = stage.tile([P, 3 * QF], f16, tag="C")
                    for q in range(3):
                        T = mm_tile(2 * (q + 1))
                        nc.scalar.copy(out=C[:, q * QF : (q + 1) * QF], in_=T)
                    # fp16 min-folds: 3072 -> 1536 -> 768 -> 384 -> [128,1]
                    F1 = folds.tile([P, 1536], f16, tag="F1")
                    nc.vector.tensor_tensor(
                        F1, C[:, 0:1536], C[:, 1536:3072], mybir.AluOpType.min
                    )
                    F2 = folds.tile([P, 768], f16, tag="F2")
                    nc.vector.tensor_tensor(
                        F2, F1[:, 0:768], F1[:, 768:1536], mybir.AluOpType.min
                    )
                    F3 = folds.tile([P, 384], f16, tag="F3")
                    nc.vector.tensor_tensor(
                        F3, F2[:, 0:384], F2[:, 384:768], mybir.AluOpType.min
                    )
                    nc.vector.tensor_reduce(
                        out=Mb[:, it : it + 1], in_=F3,
                        axis=mybir.AxisListType.X, op=mybir.AluOpType.min,
                    )

            # ---------- finals ----------
            # rowmin = min(direct, staged); partial = sum over all rowmins
            tots = []
            for Md, Mb, tag in ((Md_A, Mb_A, "A"), (Md_B, Mb_B, "B")):
                Mm = singles.tile([P, NT], f32, tag=f"Mm_{tag}")
                nc.vector.tensor_tensor(Mm, Md, Mb, mybir.AluOpType.min)
                st = singles.tile([P, 1], f32, tag=f"st_{tag}")
                nc.vector.reduce_sum(out=st, in_=Mm, axis=mybir.AxisListType.X)
                tots.append(st)
            tot = singles.tile([P, 1], f32, tag="tot")
            nc.vector.tensor_add(tot, tots[0], tots[1])
            nc.sync.dma_start(out=out_dram[:], in_=tot)

    nc.compile()
    return nc


def _get_nc():
    global _nc_cache
    if _nc_cache is None:
        _nc_cache = _build_bass()
    return _nc_cache


def kernel(prediction, ground_truth):
    global LAST_RESULT
    pred = np.ascontiguousarray(np.asarray(prediction, dtype=np.float32))
    gtr = np.ascontiguousarray(np.asarray(ground_truth, dtype=np.float32))
    assert pred.shape == (B, N, D) and gtr.shape == (B, N, D)
    nc = _get_nc()
    in_maps = [
        {"prediction": pred[b], "ground_truth": gtr[b]} for b in range(B)
    ]
    res = run_bass_kernel_spmd(nc, in_maps, list(range(B)), trace=TRACE)
    LAST_RESULT = res
    total = sum(float(np.sum(r["partial"], dtype=np.float64)) for r in res.results)
    return np.float32(total / B)



# revision 11
# speedup vs baseline: 1.1044x; 1.1044x over previous
"""Chamfer distance (CDLoss) Trainium2 Bass kernel, v2 ("pair trick").

Problem: B=8, N=4096, D=3.
  T[b,i,j] = ||pred[b,i] - gt[b,j]||^2
  loss = (sum_bj min_i T + sum_bi min_j T) / B

Sharding: one batch per NeuronCore (8 cores, SPMD). Each core computes
partial_b [128,1] (per-partition sums of row minima for both directions);
the host sums the 128 values per core, sums cores, divides by B.

Key structure (vs v1): the PSUM drain is the bottleneck, and only DVE
(0.96 GHz) + ACT (1.2 GHz) can read PSUM. Instead of materializing raw
distances T[i,j], the PE computes *pair* half-sums and half-diffs over
adjacent gt columns:
    HS[i,j'] = 0.5*(T[i,2j'] + T[i,2j'+1])
    HD[i,j'] = 0.5*(T[i,2j'] - T[i,2j'+1])
(both are bilinear in the inputs, so they come straight out of a matmul
with pre-combined moving operands). Then
    min(T[i,2j'], T[i,2j'+1]) = HS - |HD|
so ACT does Abs(HD) (PSUM->SBUF) and DVE does a single fused
tensor_tensor_reduce: (HS - |HD|) with min-accumulate -> each engine
touches only HALF of the N^2 elements. Per 128x4096 row-tile: ACT ~2.1us,
DVE ~2.4us, PE ~1.7us, all overlapped; 64 row-tiles over the two passes.

Matmuls run in fp32r (K=5 contraction: [x,y,z,||x||^2,1] lifted vectors),
which the PE processes at 1 column/cycle for moving width >= 256 --
no bf16 splitting needed, so preprocessing is ~15 small DVE ops plus
row-flatten DMAs.

TTR dummy `out` buffers and ACT staging buffers rotate (x2) to avoid
WAW serialization (~190ns/instr otherwise); min-accumulators land in
distinct collector columns per unit for the same reason.
"""

import numpy as np

import concourse.bacc as bacc
import concourse.bass as bass
import concourse.dve_ops as _dve_ops
import concourse.tile as tile
from concourse import mybir
from concourse.bass_utils import run_bass_kernel_spmd
from concourse.dve_spec import C0, Spec, Src0, Src1, lower, minn
from concourse.dve_uop import DveOpSpec

# ---- custom DVE op: out = in0 - in1; accum_out = min(s0, min_k out) ----
# The native TENSOR_TENSOR_REDUCE ISA opcode wedges the exec unit on this
# runtime build, so the same fusion is registered through the (production-
# proven) custom-DVE ucode path instead, exactly as dve_ops.py's header
# documents for new ops. Registration is additive and in-process; row and
# sha are computed here so the per-NEFF table and instruction encoding
# stay consistent.
_SUBMIN_NAME = "SUB_MIN_REDUCE_CDK"


def _submin_ref(in0, in1, c0, c1, c2):
    b = (in0.astype(np.float32) - in1).astype(np.float32)
    return b, np.minimum(
        c0, b.reshape(b.shape[0], -1).min(axis=-1, keepdims=True)
    )


def _get_submin_op():
    for op in _dve_ops.OPS:
        if op.name == _SUBMIN_NAME:
            return op
    row = _dve_ops._CUSTOM_DVE_ROW_BASE + len(_dve_ops.OPS)
    assert row < 0x20, "custom-DVE row field is 5 bits"
    spec = Spec(body=Src0 - Src1, accum=minn, accum_init=C0, reference=_submin_ref)
    _dve_ops._SUB_OPCODE_FOR_NAME[_SUBMIN_NAME] = row
    shas = {}
    for ver in ("v3", "v4"):
        uops = lower(spec, ver=ver)
        shas[ver] = DveOpSpec(
            name=_SUBMIN_NAME, opcode=row, uops=uops, rd1_en=True
        ).sha(ver)
    op = _dve_ops.DveOp(_SUBMIN_NAME, spec, subdim=False, uops_sha=shas)
    _dve_ops.OPS.append(op)
    _dve_ops.CUSTOM_DVE_SPECS[_SUBMIN_NAME] = spec
    return op

N = 4096
D = 3
B = 8
P = 128            # SBUF/PSUM partitions
KP = N // P        # 32 points per partition in staging layout
NT = N // P        # 32 row-tiles per pass
NPAIR = N // 2     # 2048 pair columns per side
UW = 1024          # unit width in pair columns (2 PSUM banks)
NU = NPAIR // UW   # 2 units per row-tile
KROWS = 24         # bf16 3-level augmented contraction rows

f32 = mybir.dt.float32
f16 = mybir.dt.float16
bf16 = mybir.dt.bfloat16

BIG = 3.0e38       # min-reduce init

TRACE = False
LAST_RESULT = None

_nc_cache = None


def _build_bass():
    submin = _get_submin_op()
    nc = bacc.Bacc(
        "TRN2", target_bir_lowering=False, debug=False, num_devices=B,
        num_swdge_queues=4,
    )
    pred = nc.declare_dram_parameter("prediction", [N, D], f32, isOutput=False)
    gt = nc.declare_dram_parameter("ground_truth", [N, D], f32, isOutput=False)
    out_dram = nc.declare_dram_parameter("partial", [P, 1], f32, isOutput=True)

    with tile.TileContext(nc) as tc:
        with (
            tc.tile_pool(name="singles", bufs=1) as singles,
            tc.tile_pool(name="work", bufs=2) as work,
            tc.tile_pool(name="psum", bufs=1, space="PSUM") as psum,
        ):
            # ---------- preprocessing ----------
            # Per side X, fp32 staging [128, *] layouts (point n = p*32+k),
            # pair combines in fp32, then 3-level bf16 splits (h/m/l carry
            # ~24 mantissa bits -> fp32-grade dot products at full bf16 PE
            # rate; fp32r was measured at tf32-class precision on HW, which
            # biases noisy minima far past the error budget).
            # Operand row pairing (S row r pairs with HS/HD row r):
            #   0-2:(xh,Yh) 3-5:(xh,Ym) 6-8:(xm,Yh) 9-11:(xh,Yl)
            #   12-14:(xl,Yh) 15-17:(xm,Ym)   [ml/lm/ll dropped, ~2^-27]
            #   18-20:(1, Yn h/m/l)  21-23:(xn h/m/l, ones/zeros)
            # where Y = -(x'_e + x'_o) for HS, -(x'_e - x'_o) for HD and
            # Yn = 0.5(n_e +/- n_o) splits; HS ones rows = 1, HD ones = 0.
            flat_engines = [nc.sync, nc.scalar, nc.gpsimd]
            flat_i = [0]

            def flat(dst, r, src2d, np_=P):
                eng = flat_engines[flat_i[0] % len(flat_engines)]
                flat_i[0] += 1
                eng.dma_start(
                    out=dst[r : r + 1, :].rearrange("r (p k) -> r p k", p=np_),
                    in_=src2d,
                )

            def rowcopy(dst, r0, r1, src_r0):
                eng = flat_engines[flat_i[0] % len(flat_engines)]
                flat_i[0] += 1
                eng.dma_start(
                    out=dst[r0:r1, :], in_=dst[src_r0 : src_r0 + (r1 - r0), :]
                )

            ones32 = singles.tile([P, KP], bf16, tag="ones32")
            nc.vector.memset(ones32, 1.0)
            zeros16 = singles.tile([P, KP // 2], bf16, tag="zeros16")
            nc.vector.memset(zeros16, 0.0)

            def split3(val, pfx, shape):
                h = work.tile(shape, bf16, tag=f"{pfx}h")
                nc.vector.tensor_copy(out=h, in_=val)
                h32 = work.tile(shape, f32, tag=f"{pfx}h32")
                nc.vector.tensor_copy(out=h32, in_=h)
                r1 = work.tile(shape, f32, tag=f"{pfx}r1")
                nc.vector.tensor_sub(r1, val, h32)
                m = work.tile(shape, bf16, tag=f"{pfx}m")
                nc.vector.tensor_copy(out=m, in_=r1)
                m32 = work.tile(shape, f32, tag=f"{pfx}m32")
                nc.vector.tensor_copy(out=m32, in_=m)
                r2 = work.tile(shape, f32, tag=f"{pfx}r2")
                nc.vector.tensor_sub(r2, r1, m32)
                l = work.tile(shape, bf16, tag=f"{pfx}l")
                nc.vector.tensor_copy(out=l, in_=r2)
                return h, m, l

            def side(xdram, tag):
                xt = work.tile([P, KP, D], f32, tag=f"xt_{tag}")
                nc.sync.dma_start(
                    out=xt, in_=xdram[:].rearrange("(p k) d -> p k d", p=P)
                )
                xr = work.tile([P, D, KP], f32, tag=f"xr_{tag}")
                nc.vector.tensor_copy(out=xr, in_=xt[:].rearrange("p k d -> p d k"))
                sq = work.tile([P, D, KP], f32, tag=f"sq_{tag}")
                nc.vector.tensor_mul(sq, xr, xr)
                n32 = work.tile([P, KP], f32, tag=f"n32_{tag}")
                nc.vector.tensor_add(n32, sq[:, 0, :], sq[:, 1, :])
                nc.vector.tensor_add(n32, n32, sq[:, 2, :])
                nh = work.tile([P, KP], f32, tag=f"nh_{tag}")
                nc.vector.tensor_scalar_mul(nh, n32, 0.5)

                # fp32 pair combines along k
                xv = xr[:].rearrange("p d (k two) -> p d k two", two=2)
                mc_hs = work.tile([P, D, KP // 2], f32, tag=f"mchs_{tag}")
                nc.vector.tensor_add(mc_hs, xv[:, :, :, 0], xv[:, :, :, 1])
                nc.vector.tensor_scalar_mul(mc_hs, mc_hs, -1.0)
                mc_hd = work.tile([P, D, KP // 2], f32, tag=f"mchd_{tag}")
                nc.vector.tensor_sub(mc_hd, xv[:, :, :, 1], xv[:, :, :, 0])
                nhv = nh[:].rearrange("p (k two) -> p k two", two=2)
                mn_hs = work.tile([P, KP // 2], f32, tag=f"mnhs_{tag}")
                nc.vector.tensor_add(mn_hs, nhv[:, :, 0], nhv[:, :, 1])
                mn_hd = work.tile([P, KP // 2], f32, tag=f"mnhd_{tag}")
                nc.vector.tensor_sub(mn_hd, nhv[:, :, 0], nhv[:, :, 1])

                # bf16 3-level splits
                xh, xm, xl = split3(xr, f"x{tag}", [P, D, KP])
                nh3 = split3(n32, f"n{tag}", [P, KP])
                hsc = split3(mc_hs, f"hsc{tag}", [P, D, KP // 2])
                hdc = split3(mc_hd, f"hdc{tag}", [P, D, KP // 2])
                hsn = split3(mn_hs, f"hsn{tag}", [P, KP // 2])
                hdn = split3(mn_hd, f"hdn{tag}", [P, KP // 2])

                S = singles.tile([KROWS, N], bf16, tag=f"S_{tag}")
                HS = singles.tile([KROWS, NPAIR], bf16, tag=f"HS_{tag}")
                HD = singles.tile([KROWS, NPAIR], bf16, tag=f"HD_{tag}")
                # stationary: unique rows then duplicates via rowcopy
                for d in range(D):
                    flat(S, 0 + d, xh[:, d, :])
                    flat(S, 6 + d, xm[:, d, :])
                    flat(S, 12 + d, xl[:, d, :])
                flat(S, 18, ones32)
                flat(S, 19, ones32)
                flat(S, 20, ones32)
                flat(S, 21, nh3[0])
                flat(S, 22, nh3[1])
                flat(S, 23, nh3[2])
                rowcopy(S, 3, 6, 0)
                rowcopy(S, 9, 12, 0)
                rowcopy(S, 15, 18, 6)
                # moving HS / HD
                for M, c3, n3, one in (
                    (HS, hsc, hsn, ones32[:, 0 : KP // 2]),
                    (HD, hdc, hdn, zeros16),
                ):
                    for d in range(D):
                        flat(M, 0 + d, c3[0][:, d, :])
                        flat(M, 3 + d, c3[1][:, d, :])
                        flat(M, 9 + d, c3[2][:, d, :])
                    flat(M, 18, n3[0])
                    flat(M, 19, n3[1])
                    flat(M, 20, n3[2])
                    flat(M, 21, one)
                    flat(M, 22, one)
                    flat(M, 23, one)
                    rowcopy(M, 6, 9, 0)
                    rowcopy(M, 12, 15, 0)
                    rowcopy(M, 15, 18, 3)
                return S, HS, HD

            S_P, HS_P, HD_P = side(pred, "p")
            S_G, HS_G, HD_G = side(gt, "g")

            # ---------- main passes ----------
            hs_ps = [psum.tile([P, UW], f32, name=f"hs{i}", tag=f"hs{i}") for i in range(2)]
            hd_ps = [psum.tile([P, UW], f32, name=f"hd{i}", tag=f"hd{i}") for i in range(2)]
            A_st = [singles.tile([P, UW], f32, name=f"A{i}", tag=f"A{i}") for i in range(2)]
            dump = [singles.tile([P, UW], f16, name=f"dump{i}", tag=f"dump{i}") for i in range(2)]

            # per-pass unit-min collectors; column g = unit it*NU+u
            Mcol_A = singles.tile([P, NT * NU], f32, tag="Mcol_A")
            Mcol_B = singles.tile([P, NT * NU], f32, tag="Mcol_B")

            for S, HS, HD, Mcol in (
                (S_P, HS_G, HD_G, Mcol_A),
                (S_G, HS_P, HD_P, Mcol_B),
            ):
                for it in range(NT):
                    lhsT = S[0:KROWS, it * P : (it + 1) * P]
                    for u in range(NU):
                        g = it * NU + u
                        hd = hd_ps[g % 2]
                        hs = hs_ps[g % 2]
                        for h in range(2):
                            nc.tensor.matmul(
                                hd[:, h * 512 : (h + 1) * 512], lhsT,
                                HD[0:KROWS, u * UW + h * 512 : u * UW + (h + 1) * 512],
                                start=True, stop=True,
                            )
                        nc.scalar.activation(
                            out=A_st[g % 2], in_=hd,
                            func=mybir.ActivationFunctionType.Abs,
                        )
                        for h in range(2):
                            nc.tensor.matmul(
                                hs[:, h * 512 : (h + 1) * 512], lhsT,
                                HS[0:KROWS, u * UW + h * 512 : u * UW + (h + 1) * 512],
                                start=True, stop=True,
                            )
                        nc.vector._custom_dve(
                            submin, out=dump[g % 2], in0=hs,
                            in1=A_st[g % 2], s0=BIG,
                            accum_out=Mcol[:, g : g + 1],
                        )

            # ---------- finals ----------
            # rowmin per tile = min over its NU unit-mins; partial = sum.
            tots = []
            for Mcol, tag in ((Mcol_A, "A"), (Mcol_B, "B")):
                Mv = Mcol[:].rearrange("p (t u) -> p t u", u=NU)
                Mm = singles.tile([P, NT], f32, tag=f"Mm_{tag}")
                nc.vector.tensor_tensor(
                    Mm, Mv[:, :, 0], Mv[:, :, 1], mybir.AluOpType.min
                )
                st = singles.tile([P, 1], f32, tag=f"st_{tag}")
                nc.vector.reduce_sum(out=st, in_=Mm, axis=mybir.AxisListType.X)
                tots.append(st)
            tot = singles.tile([P, 1], f32, tag="tot")
            nc.vector.tensor_add(tot, tots[0], tots[1])
            nc.sync.dma_start(out=out_dram[:], in_=tot)

    nc.compile()
    return nc


def _get_nc():
    global _nc_cache
    if _nc_cache is None:
        _nc_cache = _build_bass()
    return _nc_cache


def kernel(prediction, ground_truth):
    global LAST_RESULT
    pred = np.ascontiguousarray(np.asarray(prediction, dtype=np.float32))
    gtr = np.ascontiguousarray(np.asarray(ground_truth, dtype=np.float32))
    assert pred.shape == (B, N, D) and gtr.shape == (B, N, D)
    nc = _get_nc()
    in_maps = [
        {"prediction": pred[b], "ground_truth": gtr[b]} for b in range(B)
    ]
    res = run_bass_kernel_spmd(nc, in_maps, list(range(B)), trace=TRACE)
    LAST_RESULT = res
    total = sum(float(np.sum(r["partial"], dtype=np.float64)) for r in res.results)
    return np.float32(total / B)


# revision 16
# speedup vs baseline: 1.2395x; 1.1223x over previous
"""Chamfer distance (CDLoss) Trainium2 Bass kernel, v2 ("pair trick").

Problem: B=8, N=4096, D=3.
  T[b,i,j] = ||pred[b,i] - gt[b,j]||^2
  loss = (sum_bj min_i T + sum_bi min_j T) / B

Sharding: one batch per NeuronCore (8 cores, SPMD). Each core computes
partial_b [128,1] (per-partition sums of row minima for both directions);
the host sums the 128 values per core, sums cores, divides by B.

Key structure (vs v1): the PSUM drain is the bottleneck, and only DVE
(0.96 GHz) + ACT (1.2 GHz) can read PSUM. Instead of materializing raw
distances T[i,j], the PE computes *pair* half-sums and half-diffs over
adjacent gt columns:
    HS[i,j'] = 0.5*(T[i,2j'] + T[i,2j'+1])
    HD[i,j'] = 0.5*(T[i,2j'] - T[i,2j'+1])
(both are bilinear in the inputs, so they come straight out of a matmul
with pre-combined moving operands). Then
    min(T[i,2j'], T[i,2j'+1]) = HS - |HD|
so ACT does Abs(HD) (PSUM->SBUF) and DVE does a single fused
tensor_tensor_reduce: (HS - |HD|) with min-accumulate -> each engine
touches only HALF of the N^2 elements. Per 128x4096 row-tile: ACT ~2.1us,
DVE ~2.4us, PE ~1.7us, all overlapped; 64 row-tiles over the two passes.

Matmuls run in fp32r (K=5 contraction: [x,y,z,||x||^2,1] lifted vectors),
which the PE processes at 1 column/cycle for moving width >= 256 --
no bf16 splitting needed, so preprocessing is ~15 small DVE ops plus
row-flatten DMAs.

TTR dummy `out` buffers and ACT staging buffers rotate (x2) to avoid
WAW serialization (~190ns/instr otherwise); min-accumulators land in
distinct collector columns per unit for the same reason.
"""

import numpy as np

import concourse.bacc as bacc
import concourse.bass as bass
import concourse.dve_ops as _dve_ops
import concourse.tile as tile
from concourse import mybir
from concourse.bass_utils import run_bass_kernel_spmd
from concourse.dve_spec import C0, Spec, Src0, Src1, lower, minn
from concourse.dve_uop import DveOpSpec

# ---- custom DVE op: out = in0 - in1; accum_out = min(s0, min_k out) ----
# The native TENSOR_TENSOR_REDUCE ISA opcode wedges the exec unit on this
# runtime build, so the same fusion is registered through the (production-
# proven) custom-DVE ucode path instead, exactly as dve_ops.py's header
# documents for new ops. Registration is additive and in-process; row and
# sha are computed here so the per-NEFF table and instruction encoding
# stay consistent.
_SUBMIN_NAME = "SUB_MIN_REDUCE_CDK"


def _submin_ref(in0, in1, c0, c1, c2):
    b = (in0.astype(np.float32) - in1).astype(np.float32)
    return b, np.minimum(
        c0, b.reshape(b.shape[0], -1).min(axis=-1, keepdims=True)
    )


def _get_submin_op():
    for op in _dve_ops.OPS:
        if op.name == _SUBMIN_NAME:
            return op
    row = _dve_ops._CUSTOM_DVE_ROW_BASE + len(_dve_ops.OPS)
    assert row < 0x20, "custom-DVE row field is 5 bits"
    spec = Spec(body=Src0 - Src1, accum=minn, accum_init=C0, reference=_submin_ref)
    _dve_ops._SUB_OPCODE_FOR_NAME[_SUBMIN_NAME] = row
    shas = {}
    for ver in ("v3", "v4"):
        uops = lower(spec, ver=ver)
        shas[ver] = DveOpSpec(
            name=_SUBMIN_NAME, opcode=row, uops=uops, rd1_en=True
        ).sha(ver)
    op = _dve_ops.DveOp(_SUBMIN_NAME, spec, subdim=False, uops_sha=shas)
    _dve_ops.OPS.append(op)
    _dve_ops.CUSTOM_DVE_SPECS[_SUBMIN_NAME] = spec
    return op

N = 4096
D = 3
B = 8
P = 128            # SBUF/PSUM partitions
KP = N // P        # 32 points per partition in staging layout
NT = N // P        # 32 row-tiles per pass
NPAIR = N // 2     # 2048 pair columns per side
UW = 1024          # unit width in pair columns (2 PSUM banks)
NU = NPAIR // UW   # 2 units per row-tile
KROWS = 24         # bf16 3-level augmented contraction rows

f32 = mybir.dt.float32
f16 = mybir.dt.float16
bf16 = mybir.dt.bfloat16

BIG = 3.0e38       # min-reduce init

TRACE = False
LAST_RESULT = None

_nc_cache = None


def _build_bass():
    submin = _get_submin_op()
    nc = bacc.Bacc(
        "TRN2", target_bir_lowering=False, debug=False, num_devices=B,
        num_swdge_queues=4,
    )
    pred = nc.declare_dram_parameter("prediction", [N, D], f32, isOutput=False)
    gt = nc.declare_dram_parameter("ground_truth", [N, D], f32, isOutput=False)
    out_dram = nc.declare_dram_parameter("partial", [P, 1], f32, isOutput=True)

    with tile.TileContext(nc) as tc:
        with (
            tc.tile_pool(name="singles", bufs=1) as singles,
            tc.tile_pool(name="work", bufs=2) as work,
            tc.tile_pool(name="psum", bufs=1, space="PSUM") as psum,
            tc.tile_pool(name="dramsc", bufs=1, space="DRAM") as dramsc,
        ):
            # ---------- preprocessing ----------
            # Per side, fp32 staging (point n = p*32+k), fp32 pair combines,
            # one 3-level bf16 split per stack ([h|m|l] carry ~24 mantissa
            # bits -> fp32-grade dot products at full bf16 PE rate; fp32r
            # measured tf32-class on HW, far past the noisy-min error
            # budget), assembled into per-operand [128, 24, k] stacks and
            # shipped by ONE DMA per operand: HWDGE charges a fixed ~630ns
            # per DMA *instruction*, so 90 per-row flats would serialize
            # ~57us on the shared HWDGE device.
            # Row pairing (S row r multiplies M row r; Y = pair combine):
            #   0-2:(xh,Yh) 3-5:(xh,Ym) 6-8:(xh,Yl) 9-11:(xm,Yh)
            #   12-14:(xm,Ym) 15-17:(xl,Yh)         [ml/lm/ll ~2^-27 dropped]
            #   18-20:(1, Yn h/m/l)  21-23:(n h/m/l, ones(HS)/zeros(HD))
            # Pass-A-critical stacks (S_P, HS_G/HD_G) build on DVE before the
            # main loop; pass-B stacks build on otherwise-idle GPSIMD and
            # overlap pass A.
            ADD = mybir.AluOpType.add
            SUB = mybir.AluOpType.subtract
            MULT = mybir.AluOpType.mult

            def split3(E, val, pfx, shape):
                h = work.tile(shape, bf16, name=f"{pfx}h", tag=f"{pfx}h")
                E.tensor_copy(out=h, in_=val)
                h32 = work.tile(shape, f32, name=f"{pfx}h32", tag=f"{pfx}h32")
                E.tensor_copy(out=h32, in_=h)
                r1 = work.tile(shape, f32, name=f"{pfx}r1", tag=f"{pfx}r1")
                E.tensor_tensor(r1, val, h32, SUB)
                m = work.tile(shape, bf16, name=f"{pfx}m", tag=f"{pfx}m")
                E.tensor_copy(out=m, in_=r1)
                m32 = work.tile(shape, f32, name=f"{pfx}m32", tag=f"{pfx}m32")
                E.tensor_copy(out=m32, in_=m)
                r2 = work.tile(shape, f32, name=f"{pfx}r2", tag=f"{pfx}r2")
                E.tensor_tensor(r2, r1, m32, SUB)
                l = work.tile(shape, bf16, name=f"{pfx}l", tag=f"{pfx}l")
                E.tensor_copy(out=l, in_=r2)
                return h, m, l

            def base(xdram, tag, queue):
                # base4 slots 0-2: coords [d][k]; slot 3: ||x||^2
                xt = work.tile([P, KP, D], f32, name=f"xt_{tag}", tag=f"xt_{tag}")
                queue.dma_start(
                    out=xt, in_=xdram[:].rearrange("(p k) d -> p k d", p=P)
                )
                base4 = work.tile([P, D + 1, KP], f32, name=f"base4_{tag}", tag=f"base4_{tag}")
                nc.vector.tensor_copy(
                    out=base4[:, 0:D, :], in_=xt[:].rearrange("p k d -> p d k")
                )
                sq = work.tile([P, D, KP], f32, name=f"sq_{tag}", tag=f"sq_{tag}")
                nc.vector.tensor_mul(sq, base4[:, 0:D, :], base4[:, 0:D, :])
                nc.vector.tensor_add(base4[:, D, :], sq[:, 0, :], sq[:, 1, :])
                nc.vector.tensor_add(base4[:, D, :], base4[:, D, :], sq[:, 2, :])
                nh = work.tile([P, KP], f32, name=f"nh_{tag}", tag=f"nh_{tag}")
                nc.vector.tensor_scalar_mul(nh, base4[:, D, :], 0.5)
                return base4, nh

            def build_S(E, base4, tag, queue, S):
                h, m, l = split3(E, base4, f"s{tag}", [P, D + 1, KP])
                st = work.tile([P, KROWS, KP], bf16, name=f"stS_{tag}", tag=f"stS_{tag}")
                E.tensor_copy(out=st[:, 0:3, :], in_=h[:, 0:D, :])
                E.tensor_copy(out=st[:, 3:6, :], in_=h[:, 0:D, :])
                E.tensor_copy(out=st[:, 6:9, :], in_=h[:, 0:D, :])
                E.tensor_copy(out=st[:, 9:12, :], in_=m[:, 0:D, :])
                E.tensor_copy(out=st[:, 12:15, :], in_=m[:, 0:D, :])
                E.tensor_copy(out=st[:, 15:18, :], in_=l[:, 0:D, :])
                E.memset(st[:, 18:21, :], 1.0)
                E.tensor_copy(out=st[:, 21, :], in_=h[:, D, :])
                E.tensor_copy(out=st[:, 22, :], in_=m[:, D, :])
                E.tensor_copy(out=st[:, 23, :], in_=l[:, D, :])
                # Bounce through DRAM: a single SBUF->SBUF DMA cannot reorder
                # (both APs must lead with their partition dim: src is
                # point-major, dst is row-major). DRAM APs are unconstrained,
                # so stack->DRAM (point-major) then DRAM->SBUF (row-major).
                DS = dramsc.tile([KROWS, N], bf16, name=f"DS_{tag}", tag=f"DS_{tag}")
                queue.dma_start(
                    out=DS[:, :].rearrange("r (p k) -> p r k", p=P), in_=st
                )
                queue.dma_start(out=S[:, :], in_=DS[:, :])
                return S

            def build_M(E, base4, nh, tag, q0, q1, M_hs, M_hd):
                # pre slots: 0-2 HS coords -(e+o), 3 HS norm 0.5(ne+no),
                #            4-6 HD coords -(e-o), 7 HD norm 0.5(ne-no)
                pre = work.tile([P, 8, KP // 2], f32, name=f"pre_{tag}", tag=f"pre_{tag}")
                xv = base4[:, 0:D, :].rearrange(
                    "p d (k two) -> p d k two", two=2
                )
                # Pool has no tensor_scalar, so negate via tensor_tensor:
                # HS coords -(e+o) = (0-e)-o; HD coords -(e-o) = o-e.
                zz = work.tile([P, D, KP // 2], f32, name=f"zz_{tag}", tag=f"zz_{tag}")
                E.memset(zz, 0.0)
                ne = work.tile([P, D, KP // 2], f32, name=f"ne_{tag}", tag=f"ne_{tag}")
                E.tensor_tensor(ne, zz, xv[:, :, :, 0], SUB)
                E.tensor_tensor(pre[:, 0:3, :], ne, xv[:, :, :, 1], SUB)
                E.tensor_tensor(
                    pre[:, 4:7, :], xv[:, :, :, 1], xv[:, :, :, 0], SUB
                )
                nhv = nh[:].rearrange("p (k two) -> p k two", two=2)
                E.tensor_tensor(pre[:, 3, :], nhv[:, :, 0], nhv[:, :, 1], ADD)
                E.tensor_tensor(pre[:, 7, :], nhv[:, :, 0], nhv[:, :, 1], SUB)
                h, m, l = split3(E, pre, f"m{tag}", [P, 8, KP // 2])
                outs = []
                for st_tag, o0, nrow, fill, queue, M in (
                    (f"stHS_{tag}", 0, 3, 1.0, q0, M_hs),
                    (f"stHD_{tag}", 4, 7, 0.0, q1, M_hd),
                ):
                    st = work.tile([P, KROWS, KP // 2], bf16, name=st_tag, tag=st_tag)
                    E.tensor_copy(out=st[:, 0:3, :], in_=h[:, o0 : o0 + 3, :])
                    E.tensor_copy(out=st[:, 3:6, :], in_=m[:, o0 : o0 + 3, :])
                    E.tensor_copy(out=st[:, 6:9, :], in_=l[:, o0 : o0 + 3, :])
                    E.tensor_copy(out=st[:, 9:12, :], in_=h[:, o0 : o0 + 3, :])
                    E.tensor_copy(out=st[:, 12:15, :], in_=m[:, o0 : o0 + 3, :])
                    E.tensor_copy(out=st[:, 15:18, :], in_=h[:, o0 : o0 + 3, :])
                    E.tensor_copy(out=st[:, 18, :], in_=h[:, nrow, :])
                    E.tensor_copy(out=st[:, 19, :], in_=m[:, nrow, :])
                    E.tensor_copy(out=st[:, 20, :], in_=l[:, nrow, :])
                    E.memset(st[:, 21:24, :], fill)
                    DM = dramsc.tile(
                        [KROWS, NPAIR], bf16, name=f"D{st_tag}", tag=f"D{st_tag}"
                    )
                    queue.dma_start(
                        out=DM[:, :].rearrange("r (p k) -> p r k", p=P), in_=st
                    )
                    queue.dma_start(out=M[:, :], in_=DM[:, :])
                    outs.append(M)

            S_P = singles.tile([KROWS, N], bf16, name="S_P", tag="S_P")
            S_G = singles.tile([KROWS, N], bf16, name="S_G", tag="S_G")
            HS_P = singles.tile([KROWS, NPAIR], bf16, name="HS_P", tag="HS_P")
            HD_P = singles.tile([KROWS, NPAIR], bf16, name="HD_P", tag="HD_P")
            HS_G = singles.tile([KROWS, NPAIR], bf16, name="HS_G", tag="HS_G")
            HD_G = singles.tile([KROWS, NPAIR], bf16, name="HD_G", tag="HD_G")

            base_p, nh_p = base(pred, "p", nc.sync)
            base_g, nh_g = base(gt, "g", nc.scalar)
            # pass-A-critical operands on DVE
            build_S(nc.vector, base_p, "p", nc.sync, S_P)
            build_M(nc.vector, base_g, nh_g, "g", nc.scalar, nc.sync, HS_G, HD_G)
            # pass-B operands on GPSIMD (overlap pass A)
            build_S(nc.gpsimd, base_g, "g", nc.scalar, S_G)
            build_M(nc.gpsimd, base_p, nh_p, "p", nc.sync, nc.scalar, HS_P, HD_P)

            # ---------- main passes ----------
            hs_ps = [psum.tile([P, UW], f32, name=f"hs{i}", tag=f"hs{i}") for i in range(2)]
            hd_ps = [psum.tile([P, UW], f32, name=f"hd{i}", tag=f"hd{i}") for i in range(2)]
            A_st = [singles.tile([P, UW], f32, name=f"A{i}", tag=f"A{i}") for i in range(2)]
            dump = [singles.tile([P, UW], f16, name=f"dump{i}", tag=f"dump{i}") for i in range(2)]

            # per-pass unit-min collectors; column g = unit it*NU+u
            Mcol_A = singles.tile([P, NT * NU], f32, tag="Mcol_A")
            Mcol_B = singles.tile([P, NT * NU], f32, tag="Mcol_B")

            for S, HS, HD, Mcol in (
                (S_P, HS_G, HD_G, Mcol_A),
                (S_G, HS_P, HD_P, Mcol_B),
            ):
                for it in range(NT):
                    lhsT = S[0:KROWS, it * P : (it + 1) * P]
                    for u in range(NU):
                        g = it * NU + u
                        hd = hd_ps[g % 2]
                        hs = hs_ps[g % 2]
                        for h in range(2):
                            nc.tensor.matmul(
                                hd[:, h * 512 : (h + 1) * 512], lhsT,
                                HD[0:KROWS, u * UW + h * 512 : u * UW + (h + 1) * 512],
                                start=True, stop=True,
                            )
                        nc.scalar.activation(
                            out=A_st[g % 2], in_=hd,
                            func=mybir.ActivationFunctionType.Abs,
                        )
                        for h in range(2):
                            nc.tensor.matmul(
                                hs[:, h * 512 : (h + 1) * 512], lhsT,
                                HS[0:KROWS, u * UW + h * 512 : u * UW + (h + 1) * 512],
                                start=True, stop=True,
                            )
                        nc.vector._custom_dve(
                            submin, out=dump[g % 2], in0=hs,
                            in1=A_st[g % 2], s0=BIG,
                            accum_out=Mcol[:, g : g + 1],
                        )

            # ---------- finals ----------
            # rowmin per tile = min over its NU unit-mins; partial = sum.
            tots = []
            for Mcol, tag in ((Mcol_A, "A"), (Mcol_B, "B")):
                Mv = Mcol[:].rearrange("p (t u) -> p t u", u=NU)
                Mm = singles.tile([P, NT], f32, tag=f"Mm_{tag}")
                nc.vector.tensor_tensor(
                    Mm, Mv[:, :, 0], Mv[:, :, 1], mybir.AluOpType.min
                )
                st = singles.tile([P, 1], f32, tag=f"st_{tag}")
                nc.vector.reduce_sum(out=st, in_=Mm, axis=mybir.AxisListType.X)
                tots.append(st)
            tot = singles.tile([P, 1], f32, tag="tot")
            nc.vector.tensor_add(tot, tots[0], tots[1])
            nc.sync.dma_start(out=out_dram[:], in_=tot)

    nc.compile()
    return nc


def _get_nc():
    global _nc_cache
    if _nc_cache is None:
        _nc_cache = _build_bass()
    return _nc_cache


def kernel(prediction, ground_truth):
    global LAST_RESULT
    pred = np.ascontiguousarray(np.asarray(prediction, dtype=np.float32))
    gtr = np.ascontiguousarray(np.asarray(ground_truth, dtype=np.float32))
    assert pred.shape == (B, N, D) and gtr.shape == (B, N, D)
    nc = _get_nc()
    in_maps = [
        {"prediction": pred[b], "ground_truth": gtr[b]} for b in range(B)
    ]
    res = run_bass_kernel_spmd(nc, in_maps, list(range(B)), trace=TRACE)
    LAST_RESULT = res
    total = sum(float(np.sum(r["partial"], dtype=np.float64)) for r in res.results)
    return np.float32(total / B)


# revision 21
# speedup vs baseline: 1.2704x; 1.0249x over previous
"""Chamfer distance (CDLoss) Trainium2 Bass kernel, v2 ("pair trick").

Problem: B=8, N=4096, D=3.
  T[b,i,j] = ||pred[b,i] - gt[b,j]||^2
  loss = (sum_bj min_i T + sum_bi min_j T) / B

Sharding: one batch per NeuronCore (8 cores, SPMD). Each core computes
partial_b [128,1] (per-partition sums of row minima for both directions);
the host sums the 128 values per core, sums cores, divides by B.

Key structure (vs v1): the PSUM drain is the bottleneck, and only DVE
(0.96 GHz) + ACT (1.2 GHz) can read PSUM. Instead of materializing raw
distances T[i,j], the PE computes *pair* half-sums and half-diffs over
adjacent gt columns:
    HS[i,j'] = 0.5*(T[i,2j'] + T[i,2j'+1])
    HD[i,j'] = 0.5*(T[i,2j'] - T[i,2j'+1])
(both are bilinear in the inputs, so they come straight out of a matmul
with pre-combined moving operands). Then
    min(T[i,2j'], T[i,2j'+1]) = HS - |HD|
so ACT does Abs(HD) (PSUM->SBUF) and DVE does a single fused
tensor_tensor_reduce: (HS - |HD|) with min-accumulate -> each engine
touches only HALF of the N^2 elements. Per 128x4096 row-tile: ACT ~2.1us,
DVE ~2.4us, PE ~1.7us, all overlapped; 64 row-tiles over the two passes.

Matmuls run in fp32r (K=5 contraction: [x,y,z,||x||^2,1] lifted vectors),
which the PE processes at 1 column/cycle for moving width >= 256 --
no bf16 splitting needed, so preprocessing is ~15 small DVE ops plus
row-flatten DMAs.

TTR dummy `out` buffers and ACT staging buffers rotate (x2) to avoid
WAW serialization (~190ns/instr otherwise); min-accumulators land in
distinct collector columns per unit for the same reason.
"""

import numpy as np

import concourse.bacc as bacc
import concourse.bass as bass
import concourse.dve_ops as _dve_ops
import concourse.tile as tile
from concourse import mybir
from concourse.bass_utils import run_bass_kernel_spmd
from concourse.dve_spec import C0, Spec, Src0, Src1, lower, minn
from concourse.dve_uop import DveOpSpec

# ---- custom DVE op: out = in0 - in1; accum_out = min(s0, min_k out) ----
# The native TENSOR_TENSOR_REDUCE ISA opcode wedges the exec unit on this
# runtime build, so the same fusion is registered through the (production-
# proven) custom-DVE ucode path instead, exactly as dve_ops.py's header
# documents for new ops. Registration is additive and in-process; row and
# sha are computed here so the per-NEFF table and instruction encoding
# stay consistent.
_SUBMIN_NAME = "SUB_MIN_REDUCE_CDK"


def _submin_ref(in0, in1, c0, c1, c2):
    b = (in0.astype(np.float32) - in1).astype(np.float32)
    return b, np.minimum(
        c0, b.reshape(b.shape[0], -1).min(axis=-1, keepdims=True)
    )


def _get_submin_op():
    for op in _dve_ops.OPS:
        if op.name == _SUBMIN_NAME:
            return op
    row = _dve_ops._CUSTOM_DVE_ROW_BASE + len(_dve_ops.OPS)
    assert row < 0x20, "custom-DVE row field is 5 bits"
    spec = Spec(body=Src0 - Src1, accum=minn, accum_init=C0, reference=_submin_ref)
    _dve_ops._SUB_OPCODE_FOR_NAME[_SUBMIN_NAME] = row
    shas = {}
    for ver in ("v3", "v4"):
        uops = lower(spec, ver=ver)
        shas[ver] = DveOpSpec(
            name=_SUBMIN_NAME, opcode=row, uops=uops, rd1_en=True
        ).sha(ver)
    op = _dve_ops.DveOp(_SUBMIN_NAME, spec, subdim=False, uops_sha=shas)
    _dve_ops.OPS.append(op)
    _dve_ops.CUSTOM_DVE_SPECS[_SUBMIN_NAME] = spec
    return op

N = 4096
D = 3
B = 8
P = 128            # SBUF/PSUM partitions
KP = N // P        # 32 points per partition in staging layout
NT = N // P        # 32 row-tiles per pass
NPAIR = N // 2     # 2048 pair columns per side
UW = 1024          # unit width in pair columns (2 PSUM banks)
NU = NPAIR // UW   # 2 units per row-tile
KROWS = 24         # bf16 3-level augmented contraction rows

f32 = mybir.dt.float32
f16 = mybir.dt.float16
bf16 = mybir.dt.bfloat16

BIG = 3.0e38       # min-reduce init

TRACE = False
LAST_RESULT = None

_nc_cache = None


def _build_bass():
    submin = _get_submin_op()
    nc = bacc.Bacc(
        "TRN2", target_bir_lowering=False, debug=False, num_devices=B,
        num_swdge_queues=4,
    )
    pred = nc.declare_dram_parameter("prediction", [N, D], f32, isOutput=False)
    gt = nc.declare_dram_parameter("ground_truth", [N, D], f32, isOutput=False)
    out_dram = nc.declare_dram_parameter("partial", [P, 1], f32, isOutput=True)

    with tile.TileContext(nc) as tc:
        with (
            tc.tile_pool(name="singles", bufs=1) as singles,
            tc.tile_pool(name="work", bufs=2) as work,
            tc.tile_pool(name="psum", bufs=1, space="PSUM") as psum,
            tc.tile_pool(name="dramsc", bufs=1, space="DRAM") as dramsc,
        ):
            # ---------- preprocessing ----------
            # Per side, fp32 staging (point n = p*32+k), fp32 pair combines,
            # one 3-level bf16 split per stack ([h|m|l] carry ~24 mantissa
            # bits -> fp32-grade dot products at full bf16 PE rate; fp32r
            # measured tf32-class on HW, far past the noisy-min error
            # budget), assembled into per-operand [128, 24, k] stacks and
            # shipped by ONE DMA per operand: HWDGE charges a fixed ~630ns
            # per DMA *instruction*, so 90 per-row flats would serialize
            # ~57us on the shared HWDGE device.
            # Row pairing (S row r multiplies M row r; Y = pair combine):
            #   0-2:(xh,Yh) 3-5:(xh,Ym) 6-8:(xh,Yl) 9-11:(xm,Yh)
            #   12-14:(xm,Ym) 15-17:(xl,Yh)         [ml/lm/ll ~2^-27 dropped]
            #   18-20:(1, Yn h/m/l)  21-23:(n h/m/l, ones(HS)/zeros(HD))
            # Pass-A-critical stacks (S_P, HS_G/HD_G) build on DVE before the
            # main loop; pass-B stacks build on otherwise-idle GPSIMD and
            # overlap pass A.
            ADD = mybir.AluOpType.add
            SUB = mybir.AluOpType.subtract
            MULT = mybir.AluOpType.mult

            def split3(E, val, pfx, shape):
                # engines subtract mixed f32 - bf16 directly, so the split
                # chain is 5 ops, not 7
                h = work.tile(shape, bf16, name=f"{pfx}h", tag=f"{pfx}h")
                E.tensor_copy(out=h, in_=val)
                r1 = work.tile(shape, f32, name=f"{pfx}r1", tag=f"{pfx}r1")
                E.tensor_tensor(r1, val, h, SUB)
                m = work.tile(shape, bf16, name=f"{pfx}m", tag=f"{pfx}m")
                E.tensor_copy(out=m, in_=r1)
                r2 = work.tile(shape, f32, name=f"{pfx}r2", tag=f"{pfx}r2")
                E.tensor_tensor(r2, r1, m, SUB)
                l = work.tile(shape, bf16, name=f"{pfx}l", tag=f"{pfx}l")
                E.tensor_copy(out=l, in_=r2)
                return h, m, l

            def base(xdram, tag, queue):
                # base4 slots 0-2: coords [d][k]; slot 3: ||x||^2
                xt = work.tile([P, KP, D], f32, name=f"xt_{tag}", tag=f"xt_{tag}")
                queue.dma_start(
                    out=xt, in_=xdram[:].rearrange("(p k) d -> p k d", p=P)
                )
                base4 = work.tile([P, D + 1, KP], f32, name=f"base4_{tag}", tag=f"base4_{tag}")
                nc.vector.tensor_copy(
                    out=base4[:, 0:D, :], in_=xt[:].rearrange("p k d -> p d k")
                )
                sq = work.tile([P, D, KP], f32, name=f"sq_{tag}", tag=f"sq_{tag}")
                nc.vector.tensor_mul(sq, base4[:, 0:D, :], base4[:, 0:D, :])
                nc.vector.tensor_add(base4[:, D, :], sq[:, 0, :], sq[:, 1, :])
                nc.vector.tensor_add(base4[:, D, :], base4[:, D, :], sq[:, 2, :])
                nh = work.tile([P, KP], f32, name=f"nh_{tag}", tag=f"nh_{tag}")
                nc.vector.tensor_scalar_mul(nh, base4[:, D, :], 0.5)
                return base4, nh

            def build_S(E, base4, tag, queue, S):
                h, m, l = split3(E, base4, f"s{tag}", [P, D + 1, KP])
                st = work.tile([P, KROWS, KP], bf16, name=f"stS_{tag}", tag=f"stS_{tag}")
                E.tensor_copy(out=st[:, 0:3, :], in_=h[:, 0:D, :])
                E.tensor_copy(out=st[:, 3:6, :], in_=h[:, 0:D, :])
                E.tensor_copy(out=st[:, 6:9, :], in_=h[:, 0:D, :])
                E.tensor_copy(out=st[:, 9:12, :], in_=m[:, 0:D, :])
                E.tensor_copy(out=st[:, 12:15, :], in_=m[:, 0:D, :])
                E.tensor_copy(out=st[:, 15:18, :], in_=l[:, 0:D, :])
                E.memset(st[:, 18:21, :], 1.0)
                E.tensor_copy(out=st[:, 21, :], in_=h[:, D, :])
                E.tensor_copy(out=st[:, 22, :], in_=m[:, D, :])
                E.tensor_copy(out=st[:, 23, :], in_=l[:, D, :])
                # Bounce through DRAM: a single SBUF->SBUF DMA cannot reorder
                # (both APs must lead with their partition dim: src is
                # point-major, dst is row-major). DRAM APs are unconstrained,
                # so stack->DRAM (point-major) then DRAM->SBUF (row-major).
                DS = dramsc.tile([KROWS, N], bf16, name=f"DS_{tag}", tag=f"DS_{tag}")
                queue.dma_start(
                    out=DS[:, :].rearrange("r (p k) -> p r k", p=P), in_=st
                )
                queue.dma_start(out=S[:, :], in_=DS[:, :])
                return S

            def build_M(E, base4, nh, tag, q0, q1, M_hs, M_hd):
                # pre slots: 0-2 HS coords -(e+o), 3 HS norm 0.5(ne+no),
                #            4-6 HD coords -(e-o), 7 HD norm 0.5(ne-no)
                pre = work.tile([P, 8, KP // 2], f32, name=f"pre_{tag}", tag=f"pre_{tag}")
                xv = base4[:, 0:D, :].rearrange(
                    "p d (k two) -> p d k two", two=2
                )
                # Pool has no tensor_scalar, so negate via tensor_tensor:
                # HS coords -(e+o) = (0-e)-o; HD coords -(e-o) = o-e.
                zz = work.tile([P, D, KP // 2], f32, name=f"zz_{tag}", tag=f"zz_{tag}")
                E.memset(zz, 0.0)
                ne = work.tile([P, D, KP // 2], f32, name=f"ne_{tag}", tag=f"ne_{tag}")
                E.tensor_tensor(ne, zz, xv[:, :, :, 0], SUB)
                E.tensor_tensor(pre[:, 0:3, :], ne, xv[:, :, :, 1], SUB)
                E.tensor_tensor(
                    pre[:, 4:7, :], xv[:, :, :, 1], xv[:, :, :, 0], SUB
                )
                nhv = nh[:].rearrange("p (k two) -> p k two", two=2)
                E.tensor_tensor(pre[:, 3, :], nhv[:, :, 0], nhv[:, :, 1], ADD)
                E.tensor_tensor(pre[:, 7, :], nhv[:, :, 0], nhv[:, :, 1], SUB)
                h, m, l = split3(E, pre, f"m{tag}", [P, 8, KP // 2])
                outs = []
                for st_tag, o0, nrow, fill, queue, M in (
                    (f"stHS_{tag}", 0, 3, 1.0, q0, M_hs),
                    (f"stHD_{tag}", 4, 7, 0.0, q1, M_hd),
                ):
                    st = work.tile([P, KROWS, KP // 2], bf16, name=st_tag, tag=st_tag)
                    E.tensor_copy(out=st[:, 0:3, :], in_=h[:, o0 : o0 + 3, :])
                    E.tensor_copy(out=st[:, 3:6, :], in_=m[:, o0 : o0 + 3, :])
                    E.tensor_copy(out=st[:, 6:9, :], in_=l[:, o0 : o0 + 3, :])
                    E.tensor_copy(out=st[:, 9:12, :], in_=h[:, o0 : o0 + 3, :])
                    E.tensor_copy(out=st[:, 12:15, :], in_=m[:, o0 : o0 + 3, :])
                    E.tensor_copy(out=st[:, 15:18, :], in_=h[:, o0 : o0 + 3, :])
                    E.tensor_copy(out=st[:, 18, :], in_=h[:, nrow, :])
                    E.tensor_copy(out=st[:, 19, :], in_=m[:, nrow, :])
                    E.tensor_copy(out=st[:, 20, :], in_=l[:, nrow, :])
                    E.memset(st[:, 21:24, :], fill)
                    DM = dramsc.tile(
                        [KROWS, NPAIR], bf16, name=f"D{st_tag}", tag=f"D{st_tag}"
                    )
                    queue.dma_start(
                        out=DM[:, :].rearrange("r (p k) -> p r k", p=P), in_=st
                    )
                    queue.dma_start(out=M[:, :], in_=DM[:, :])
                    outs.append(M)

            S_P = singles.tile([KROWS, N], bf16, name="S_P", tag="S_P")
            S_G = singles.tile([KROWS, N], bf16, name="S_G", tag="S_G")
            HS_P = singles.tile([KROWS, NPAIR], bf16, name="HS_P", tag="HS_P")
            HD_P = singles.tile([KROWS, NPAIR], bf16, name="HD_P", tag="HD_P")
            HS_G = singles.tile([KROWS, NPAIR], bf16, name="HS_G", tag="HS_G")
            HD_G = singles.tile([KROWS, NPAIR], bf16, name="HD_G", tag="HD_G")

            base_p, nh_p = base(pred, "p", nc.sync)
            base_g, nh_g = base(gt, "g", nc.scalar)
            # pass-A-critical operands: stat-P on GPSIMD runs in parallel
            # with moving-G on DVE.
            build_S(nc.gpsimd, base_p, "p", nc.sync, S_P)
            build_M(nc.vector, base_g, nh_g, "g", nc.scalar, nc.sync, HS_G, HD_G)

            # ---------- main passes ----------
            hs_ps = [psum.tile([P, UW], f32, name=f"hs{i}", tag=f"hs{i}") for i in range(2)]
            hd_ps = [psum.tile([P, UW], f32, name=f"hd{i}", tag=f"hd{i}") for i in range(2)]
            A_st = [singles.tile([P, UW], f32, name=f"A{i}", tag=f"A{i}") for i in range(2)]
            dump = [singles.tile([P, UW], f16, name=f"dump{i}", tag=f"dump{i}") for i in range(2)]

            # unit-min collector; pass A in cols 0-63, pass B in 64-127
            McolAB = singles.tile([P, 2 * NT * NU], f32, tag="McolAB")
            Mcol_A = McolAB[:, 0 : NT * NU]
            Mcol_B = McolAB[:, NT * NU : 2 * NT * NU]

            passes = (
                (S_P, HS_G, HD_G, Mcol_A),
                (S_G, HS_P, HD_P, Mcol_B),
            )
            for pass_i, (S, HS, HD, Mcol) in enumerate(passes):
                if pass_i == 1:
                    # Emit pass-B operand builds here: GPSIMD gates on an
                    # early pass-A staging buffer so its stack DMAs stay off
                    # the shared HWDGE during the pass-A-critical prefix.
                    # Pool executes in order, so one gated copy (unit 0's
                    # collector column, written exactly once) holds all GP
                    # side-B work until pass A is underway.
                    gate = work.tile([P, 1], f32, name="gate", tag="gate")
                    nc.gpsimd.tensor_copy(out=gate, in_=McolAB[:, 0:1])
                    build_S(nc.gpsimd, base_g, "g", nc.scalar, S_G)
                    build_M(
                        nc.gpsimd, base_p, nh_p, "p", nc.sync, nc.scalar,
                        HS_P, HD_P,
                    )
                for it in range(NT):
                    lhsT = S[0:KROWS, it * P : (it + 1) * P]
                    for u in range(NU):
                        g = it * NU + u
                        hd = hd_ps[g % 2]
                        hs = hs_ps[g % 2]
                        for h in range(2):
                            nc.tensor.matmul(
                                hd[:, h * 512 : (h + 1) * 512], lhsT,
                                HD[0:KROWS, u * UW + h * 512 : u * UW + (h + 1) * 512],
                                start=True, stop=True,
                            )
                        nc.scalar.activation(
                            out=A_st[g % 2], in_=hd,
                            func=mybir.ActivationFunctionType.Abs,
                        )
                        for h in range(2):
                            nc.tensor.matmul(
                                hs[:, h * 512 : (h + 1) * 512], lhsT,
                                HS[0:KROWS, u * UW + h * 512 : u * UW + (h + 1) * 512],
                                start=True, stop=True,
                            )
                        nc.vector._custom_dve(
                            submin, out=dump[g % 2], in0=hs,
                            in1=A_st[g % 2], s0=BIG,
                            accum_out=Mcol[:, g : g + 1],
                        )

            # ---------- finals ----------
            # rowmin per tile = min over its NU unit-mins; partial = sum of
            # all tile rowmins from both passes in one reduce.
            Mv = McolAB[:].rearrange("p (t u) -> p t u", u=NU)
            Mm = singles.tile([P, 2 * NT], f32, tag="Mm")
            nc.vector.tensor_tensor(
                Mm, Mv[:, :, 0], Mv[:, :, 1], mybir.AluOpType.min
            )
            tot = singles.tile([P, 1], f32, tag="tot")
            nc.vector.reduce_sum(out=tot, in_=Mm, axis=mybir.AxisListType.X)
            nc.sync.dma_start(out=out_dram[:], in_=tot)

    nc.compile()
    return nc


def _get_nc():
    global _nc_cache
    if _nc_cache is None:
        _nc_cache = _build_bass()
    return _nc_cache


def kernel(prediction, ground_truth):
    global LAST_RESULT
    pred = np.ascontiguousarray(np.asarray(prediction, dtype=np.float32))
    gtr = np.ascontiguousarray(np.asarray(ground_truth, dtype=np.float32))
    assert pred.shape == (B, N, D) and gtr.shape == (B, N, D)
    nc = _get_nc()
    in_maps = [
        {"prediction": pred[b], "ground_truth": gtr[b]} for b in range(B)
    ]
    res = run_bass_kernel_spmd(nc, in_maps, list(range(B)), trace=TRACE)
    LAST_RESULT = res
    total = sum(float(np.sum(r["partial"], dtype=np.float64)) for r in res.results)
    return np.float32(total / B)
